# revision 4
# baseline (speedup 1.0000x reference)
"""2-layer GCN (GCNConv -> ReLU -> GCNConv) on 8 TRN2 NeuronCores.

Sharding: output nodes are split into 8 shards (one per core); edges are
partitioned by destination shard so each core owns the scatter-add for its
nodes. Hidden features of source nodes are exchanged with an on-device
AllGather between the per-shard transform and the aggregation.

Per-core pipeline (single SPMD Bass program, all cores identical; per-core
behavior comes from per-core input data):
  1. transform: h' = dinv * (x @ W1) for the own shard (TensorE, fp32),
     using a host-pretransposed x so no on-device transpose is needed.
  2. AllGather h' -> full 50176-row table in each core's DRAM.
  3. aggregation: edges are pre-sorted by destination block (49 blocks of
     128 dst nodes per core, balanced by in-degree via a host-side node
     relabeling). Source rows are fetched with dma_gather (512B rows,
     <=1024 indices per call - the SWDGE ring limit), landing edge-major in
     SBUF chunks of 128. A one-hot matrix S (built on VectorE via
     iota == dst_local) folds each chunk into the block's PSUM accumulator
     on TensorE. dma_gather indices are int16, so each block's slots are
     segregated into a "lo" (<32768) and "hi" region addressed from a
     shifted table base.
  4. out = dinv * (agg + h'_own) + bias (the self-loop and symmetric
     normalization fold into two dinv scalings); ReLU on ScalarE.
  5. repeat 1-4 with W2/b2; z shard is DMA'd out and un-permuted on host.
"""

import os

import numpy as np

P = 128
N_CORES = 8
N_NODES = 50000
IN_DIM = 256
HID = 128
NB = 49
SHARD = NB * P  # 6272
NPAD = N_CORES * SHARD  # 50176
HI_BASE = 32768
MAX_CALL = 1024

LAST_EXEC_NS = None
LAST_RESULT = None


def _wrap16(flat, ncols):
    w = np.zeros((16, ncols), np.uint16)
    n = len(flat)
    w[np.arange(n) % 16, np.arange(n) // 16] = flat
    return np.tile(w, (8, 1)).view(np.int16)


def _host_prep(x, edge_index, W1, b1, W2, b2):
    src = np.asarray(edge_index[0], dtype=np.int64)
    dst = np.asarray(edge_index[1], dtype=np.int64)
    x = np.asarray(x, dtype=np.float32)

    indeg = np.bincount(dst, minlength=N_NODES)
    deg = indeg + 1.0
    dinv = (1.0 / np.sqrt(deg)).astype(np.float32)

    # per-shard relabeling: deal nodes (by in-degree desc) round-robin into
    # the 49 dst blocks so block edge counts are balanced across cores.
    old_shard = N_NODES // N_CORES
    new_of_old = np.empty(N_NODES, np.int64)
    old_of_new = np.full(NPAD, -1, np.int64)
    for c in range(N_CORES):
        olds = np.arange(c * old_shard, (c + 1) * old_shard)
        order = olds[np.argsort(-indeg[olds], kind="stable")]
        pos_in_block = np.arange(len(order)) // NB
        block = np.arange(len(order)) % NB
        news = c * SHARD + block * P + pos_in_block
        new_of_old[order] = news
        old_of_new[news] = order

    src_n = new_of_old[src]
    dst_n = new_of_old[dst]

    core_of_dst = dst_n // SHARD
    lo_lists = [[None] * NB for _ in range(N_CORES)]
    hi_lists = [[None] * NB for _ in range(N_CORES)]
    for c in range(N_CORES):
        m = core_of_dst == c
        s, d = src_n[m], dst_n[m] - c * SHARD
        b = d // P
        r = d % P
        hi = s >= HI_BASE
        for bb in range(NB):
            mb = b == bb
            mlo = mb & ~hi
            mhi = mb & hi
            lo_lists[c][bb] = (s[mlo], r[mlo])
            hi_lists[c][bb] = (s[mhi] - HI_BASE, r[mhi])

    C_lo = np.zeros(NB, np.int64)
    C_hi = np.zeros(NB, np.int64)
    for b in range(NB):
        for c in range(N_CORES):
            C_lo[b] = max(C_lo[b], (len(lo_lists[c][b][0]) + P - 1) // P)
            C_hi[b] = max(C_hi[b], (len(hi_lists[c][b][0]) + P - 1) // P)
    C_blk = C_lo + C_hi
    NC = int(C_blk.sum())

    idx_mats, d_mats = [], []
    for c in range(N_CORES):
        idx_flat = np.zeros(NC * P, np.int64)
        dloc = np.full((P, NC), -1.0, np.float32)
        chunk0 = 0
        for b in range(NB):
            for lists, C in ((lo_lists, C_lo[b]), (hi_lists, C_hi[b])):
                s, r = lists[c][b]
                n = len(s)
                base = chunk0 * P
                idx_flat[base : base + n] = s
                j = np.arange(n)
                dloc[j % P, chunk0 + j // P] = r
                chunk0 += C
        idx_mats.append(_wrap16(idx_flat, NC * 8))
        d_mats.append(dloc)

    calls = []
    chunk0 = 0
    for b in range(NB):
        for C, is_hi in ((C_lo[b], False), (C_hi[b], True)):
            left = int(C)
            at = chunk0
            while left > 0:
                k = min(left, MAX_CALL // P)
                calls.append((at, k, is_hi))
                at += k
                left -= k
            chunk0 += int(C)

    xs, dinvs = [], []
    for c in range(N_CORES):
        xc = np.zeros((SHARD, IN_DIM), np.float32)
        dc = np.ones((SHARD,), np.float32)
        sel = old_of_new[c * SHARD : (c + 1) * SHARD]
        real = sel >= 0
        xc[real] = x[sel[real]]
        dc[real] = dinv[sel[real]]
        dw = dc.reshape(NB, P).T.copy()
        xT = np.ascontiguousarray(xc.T.reshape(2, P, SHARD).transpose(1, 0, 2))
        xs.append(xT.reshape(P, 2 * SHARD))
        dinvs.append(dw)

    iota = np.tile(np.arange(P, dtype=np.float32)[None, :], (P, 1))
    ident = np.eye(P, dtype=np.float32)
    b1r = np.tile(np.asarray(b1, np.float32)[None, :], (P, 1))
    b2r = np.tile(np.asarray(b2, np.float32)[None, :], (P, 1))

    in_maps = []
    for c in range(N_CORES):
        in_maps.append(
            {
                "x": xs[c],
                "gidx": idx_mats[c],
                "dmat": d_mats[c],
                "dinv": dinvs[c],
                "w1": np.asarray(W1, np.float32),
                "w2": np.asarray(W2, np.float32),
                "b1r": b1r,
                "b2r": b2r,
                "iota": iota,
                "ident": ident,
            }
        )

    meta = dict(C_blk=C_blk, NC=NC, calls=calls, old_of_new=old_of_new)
    return in_maps, meta


def _build_program(meta, table_bf16=False):
    import concourse.mybir as mybir
    import concourse.tile as tile
    from concourse import bacc
    from concourse._compat import get_trn_type

    C_blk = meta["C_blk"]
    NC = meta["NC"]
    calls = meta["calls"]
    C_MAX = int(C_blk.max())
    f32 = mybir.dt.float32
    tdt = mybir.dt.bfloat16 if table_bf16 else f32

    nc = bacc.Bacc(get_trn_type() or "TRN2")
    x_in = nc.dram_tensor("x", [P, 2 * SHARD], f32, kind="ExternalInput")
    gidx = nc.dram_tensor("gidx", [P, NC * 8], mybir.dt.int16, kind="ExternalInput")
    dmat = nc.dram_tensor("dmat", [P, NC], f32, kind="ExternalInput")
    dinv_in = nc.dram_tensor("dinv", [P, NB], f32, kind="ExternalInput")
    w1_in = nc.dram_tensor("w1", [IN_DIM, HID], f32, kind="ExternalInput")
    w2_in = nc.dram_tensor("w2", [HID, HID], f32, kind="ExternalInput")
    b1_in = nc.dram_tensor("b1r", [P, HID], f32, kind="ExternalInput")
    b2_in = nc.dram_tensor("b2r", [P, HID], f32, kind="ExternalInput")
    iota_in = nc.dram_tensor("iota", [P, P], f32, kind="ExternalInput")
    ident_in = nc.dram_tensor("ident", [P, P], f32, kind="ExternalInput")
    z_out = nc.dram_tensor("z", [SHARD, HID], f32, kind="ExternalOutput")

    cc1_in = nc.dram_tensor("cc1_in", [SHARD, HID], tdt)
    table1 = nc.dram_tensor("table1", [NPAD, HID], tdt, addr_space="Shared")
    cc2_in = nc.dram_tensor("cc2_in", [SHARD, HID], tdt)
    table2 = nc.dram_tensor("table2", [NPAD, HID], tdt, addr_space="Shared")

    rg = [list(range(N_CORES))]

    with tile.TileContext(nc) as tc:
        with (
            tc.tile_pool(name="persist", bufs=1) as pp,
            tc.tile_pool(name="xt", bufs=4) as xtp,
            tc.tile_pool(name="g", bufs=3) as gp,
            tc.tile_pool(name="s", bufs=6) as sp,
            tc.tile_pool(name="ep", bufs=4) as ep,
            tc.tile_pool(name="psum", bufs=2, space="PSUM") as psp,
        ):
            idx_t = pp.tile([P, NC * 8], mybir.dt.int16)
            nc.sync.dma_start(out=idx_t[:], in_=gidx[:])
            dm_t = pp.tile([P, NC], f32)
            nc.sync.dma_start(out=dm_t[:], in_=dmat[:])
            dinv_t = pp.tile([P, NB], f32)
            nc.sync.dma_start(out=dinv_t[:], in_=dinv_in[:])
            iota_t = pp.tile([P, P], f32)
            nc.sync.dma_start(out=iota_t[:], in_=iota_in[:])
            ident_t = pp.tile([P, P], f32)
            nc.sync.dma_start(out=ident_t[:], in_=ident_in[:])
            b1_t = pp.tile([P, HID], f32)
            nc.sync.dma_start(out=b1_t[:], in_=b1_in[:])
            b2_t = pp.tile([P, HID], f32)
            nc.sync.dma_start(out=b2_t[:], in_=b2_in[:])
            w1_t = pp.tile([P, 2 * HID], f32)
            nc.sync.dma_start(
                out=w1_t[:].rearrange("p (k h) -> p k h", k=2),
                in_=w1_in[:].rearrange("(k p) h -> p k h", p=P),
            )
            w2_t = pp.tile([P, HID], f32)
            nc.sync.dma_start(out=w2_t[:], in_=w2_in[:])

            hbuf = pp.tile([P, SHARD], f32)
            x2buf = pp.tile([P, SHARD], f32)
            h2buf = pp.tile([P, SHARD], f32)

            def transform(get_lhsT, w_tiles, out_sbuf, cc_dram):
                nkt = len(w_tiles)
                for t in range(NB):
                    hp = psp.tile([P, HID], f32, tag="hp")
                    for k in range(nkt):
                        nc.tensor.matmul(
                            out=hp[:], lhsT=get_lhsT(t, k), rhs=w_tiles[k],
                            start=(k == 0), stop=(k == nkt - 1),
                        )
                    sl = out_sbuf[:, t * P : (t + 1) * P]
                    nc.vector.tensor_scalar(
                        out=sl, in0=hp[:], scalar1=dinv_t[:, t : t + 1],
                        scalar2=None, op0=mybir.AluOpType.mult,
                    )
                    if table_bf16:
                        hc = xtp.tile([P, HID], tdt, tag="hcast")
                        nc.scalar.copy(out=hc[:], in_=sl)
                        nc.sync.dma_start(
                            out=cc_dram[t * P : (t + 1) * P, :], in_=hc[:]
                        )
                    else:
                        nc.sync.dma_start(
                            out=cc_dram[t * P : (t + 1) * P, :], in_=sl
                        )

            def aggregate(table, hsrc, bias_t, relu, z_dram):
                chunk0 = 0
                ci = 0
                for b in range(NB):
                    Cb = int(C_blk[b])
                    G = gp.tile([P, C_MAX * P], tdt, tag="g")
                    G3 = G[:].rearrange("p (c d) -> p c d", d=P)
                    while ci < len(calls) and calls[ci][0] < chunk0 + Cb:
                        at, k, is_hi = calls[ci]
                        n = k * P
                        src = table[HI_BASE:, :] if is_hi else table[:, :]
                        nc.gpsimd.dma_gather(
                            G3[:, at - chunk0 : at - chunk0 + k, :],
                            src,
                            idx_t[:, at * 8 : at * 8 + n // 16],
                            n, n, HID,
                        )
                        ci += 1
                    acc = psp.tile([P, HID], f32, tag="acc")
                    for i in range(Cb):
                        S = sp.tile([P, P], tdt, tag="S")
                        nc.vector.tensor_tensor(
                            out=S[:], in0=iota_t[:],
                            in1=dm_t[
                                :, chunk0 + i : chunk0 + i + 1
                            ].to_broadcast([P, P]),
                            op=mybir.AluOpType.is_equal,
                        )
                        nc.tensor.matmul(
                            out=acc[:], lhsT=S[:], rhs=G3[:, i, :],
                            start=(i == 0), stop=(i == Cb - 1),
                        )
                    t1 = ep.tile([P, HID], f32, tag="t1")
                    nc.vector.tensor_tensor(
                        out=t1[:], in0=acc[:],
                        in1=hsrc[:, b * P : (b + 1) * P],
                        op=mybir.AluOpType.add,
                    )
                    t2 = ep.tile([P, HID], f32, tag="t2")
                    nc.vector.scalar_tensor_tensor(
                        out=t2[:], in0=t1[:],
                        scalar=dinv_t[:, b : b + 1], in1=bias_t[:],
                        op0=mybir.AluOpType.mult, op1=mybir.AluOpType.add,
                    )
                    if relu:
                        nc.scalar.activation(
                            out=x2buf[:, b * P : (b + 1) * P], in_=t2[:],
                            func=mybir.ActivationFunctionType.Relu,
                        )
                    else:
                        nc.sync.dma_start(
                            out=z_dram[b * P : (b + 1) * P, :], in_=t2[:]
                        )
                    chunk0 += Cb

            # layer 1
            xT_t = pp.tile([P, 2 * SHARD], f32)
            nc.sync.dma_start(out=xT_t[:], in_=x_in[:])
            xT3 = xT_t[:].rearrange("p (k n) -> p k n", k=2)
            transform(
                lambda t, k: xT3[:, k, t * P : (t + 1) * P],
                [w1_t[:, 0:HID], w1_t[:, HID : 2 * HID]], hbuf, cc1_in,
            )
            nc.gpsimd.collective_compute(
                "AllGather", mybir.AluOpType.bypass, replica_groups=rg,
                ins=[cc1_in[:]], outs=[table1[:]],
            )
            aggregate(table1, hbuf, b1_t, True, None)

            # layer 2
            def l2_lhsT(t, k):
                tp = psp.tile([P, P], f32, tag="tp")
                nc.tensor.transpose(
                    out=tp[:], in_=x2buf[:, t * P : (t + 1) * P],
                    identity=ident_t[:],
                )
                xT = xtp.tile([P, P], f32, tag="xT")
                nc.scalar.copy(out=xT[:], in_=tp[:])
                return xT[:]

            transform(l2_lhsT, [w2_t[:]], h2buf, cc2_in)
            nc.gpsimd.collective_compute(
                "AllGather", mybir.AluOpType.bypass, replica_groups=rg,
                ins=[cc2_in[:]], outs=[table2[:]],
            )
            aggregate(table2, h2buf, b2_t, False, z_out)

    nc.compile()
    return nc


def kernel(x, edge_index, W1, b1, W2, b2):
    global LAST_EXEC_NS, LAST_RESULT
    from concourse.bass_utils import run_bass_kernel_spmd

    trace = bool(int(os.environ.get("GCN_TRACE", "0")))
    if trace:
        try:  # NTFF profiling shim (axon images lack antenv.axon_hooks)
            _install_ntff_shim()
        except Exception:
            trace = False

    table_bf16 = bool(int(os.environ.get("GCN_BF16", "0")))
    in_maps, meta = _host_prep(x, edge_index, W1, b1, W2, b2)
    nc = _build_program(meta, table_bf16=table_bf16)
    res = run_bass_kernel_spmd(
        nc, in_maps, core_ids=list(range(N_CORES)), trace=trace
    )
    LAST_EXEC_NS = res.exec_time_ns
    LAST_RESULT = res

    old_of_new = meta["old_of_new"]
    z = np.zeros((N_NODES, HID), np.float32)
    for c in range(N_CORES):
        zc = np.asarray(res.results[c]["z"])
        sel = old_of_new[c * SHARD : (c + 1) * SHARD]
        real = sel >= 0
        z[sel[real]] = zc[real]
    return z


def _install_ntff_shim():
    import contextlib
    import ctypes
    import sys
    import types

    if "antenv.axon_hooks" in sys.modules:
        return
    lib = ctypes.CDLL("/opt/axon/libaxon_pjrt.so")
    if not hasattr(lib, "axon_start_nrt_profile"):
        raise RuntimeError("no profile symbols")
    lib.axon_start_nrt_profile.argtypes = [
        ctypes.POINTER(ctypes.c_int64),
        ctypes.c_size_t,
    ]
    lib.axon_start_nrt_profile.restype = ctypes.c_int64
    lib.axon_stop_nrt_profile.argtypes = [ctypes.c_char_p]
    lib.axon_stop_nrt_profile.restype = ctypes.c_int64

    @contextlib.contextmanager
    def _hook(output_dir, device_ids):
        import jax

        jax.devices()
        if device_ids:
            ids = (ctypes.c_int64 * len(device_ids))(*device_ids)
            rc = lib.axon_start_nrt_profile(ids, len(device_ids))
        else:
            rc = lib.axon_start_nrt_profile(None, 0)
        if rc != 0:
            raise RuntimeError(f"axon_start_nrt_profile rc={rc}")
        try:
            yield
        finally:
            lib.axon_stop_nrt_profile(str(output_dir).encode())

    mod = types.ModuleType("antenv.axon_hooks")
    mod.get_axon_ntff_profile_hook = lambda: _hook
    mod.set_axon_ntff_profile_hook = lambda h: None
    sys.modules["antenv.axon_hooks"] = mod
    import antenv

    antenv.axon_hooks = mod



# revision 8
# speedup vs baseline: 2.7717x; 2.7717x over previous
"""2-layer GCN (GCNConv -> ReLU -> GCNConv) on 8 TRN2 NeuronCores.

Sharding: output nodes are split into 8 shards (one per core); edges are
partitioned by destination shard so each core owns the scatter-add for its
nodes. Hidden features of source nodes are exchanged with an on-device
AllGather between the per-shard transform and the aggregation.

Per-core pipeline (single SPMD Bass program, all cores identical; per-core
behavior comes from per-core input data):
  1. transform: h' = dinv * (x @ W1) for the own shard (TensorE, fp32),
     using a host-pretransposed x so no on-device transpose is needed.
  2. AllGather h' -> full 50176-row table in each core's DRAM.
  3. aggregation: edges are pre-sorted by destination block (49 blocks of
     128 dst nodes per core, balanced by in-degree via a host-side node
     relabeling). Source rows are fetched with dma_gather (512B rows,
     <=1024 indices per call - the SWDGE ring limit), landing edge-major in
     SBUF chunks of 128. A one-hot matrix S (built on VectorE via
     iota == dst_local) folds each chunk into the block's PSUM accumulator
     on TensorE. dma_gather indices are int16, so each block's slots are
     segregated into a "lo" (<32768) and "hi" region addressed from a
     shifted table base.
  4. out = dinv * (agg + h'_own) + bias (the self-loop and symmetric
     normalization fold into two dinv scalings); ReLU on ScalarE.
  5. repeat 1-4 with W2/b2; z shard is DMA'd out and un-permuted on host.
"""

import os

import numpy as np

P = 128
N_CORES = 8
N_NODES = 50000
IN_DIM = 256
HID = 128
NB = 49
SHARD = NB * P  # 6272
NPAD = N_CORES * SHARD  # 50176
HI_BASE = 32768
MAX_CALL = 1024

LAST_EXEC_NS = None
LAST_RESULT = None


def _wrap16(flat, ncols):
    w = np.zeros((16, ncols), np.uint16)
    n = len(flat)
    w[np.arange(n) % 16, np.arange(n) // 16] = flat
    return np.tile(w, (8, 1)).view(np.int16)


def _host_prep(x, edge_index, W1, b1, W2, b2):
    src = np.asarray(edge_index[0], dtype=np.int64)
    dst = np.asarray(edge_index[1], dtype=np.int64)
    x = np.asarray(x, dtype=np.float32)

    indeg = np.bincount(dst, minlength=N_NODES)
    deg = indeg + 1.0
    dinv = (1.0 / np.sqrt(deg)).astype(np.float32)

    # per-shard relabeling: deal nodes (by in-degree desc) round-robin into
    # the 49 dst blocks so block edge counts are balanced across cores.
    old_shard = N_NODES // N_CORES
    new_of_old = np.empty(N_NODES, np.int64)
    old_of_new = np.full(NPAD, -1, np.int64)
    for c in range(N_CORES):
        olds = np.arange(c * old_shard, (c + 1) * old_shard)
        order = olds[np.argsort(-indeg[olds], kind="stable")]
        pos_in_block = np.arange(len(order)) // NB
        block = np.arange(len(order)) % NB
        news = c * SHARD + block * P + pos_in_block
        new_of_old[order] = news
        old_of_new[news] = order

    src_n = new_of_old[src]
    dst_n = new_of_old[dst]

    core_of_dst = dst_n // SHARD
    lo_lists = [[None] * NB for _ in range(N_CORES)]
    hi_lists = [[None] * NB for _ in range(N_CORES)]
    for c in range(N_CORES):
        m = core_of_dst == c
        s, d = src_n[m], dst_n[m] - c * SHARD
        b = d // P
        r = d % P
        hi = s >= HI_BASE
        for bb in range(NB):
            mb = b == bb
            mlo = mb & ~hi
            mhi = mb & hi
            lo_lists[c][bb] = (s[mlo], r[mlo])
            hi_lists[c][bb] = (s[mhi] - HI_BASE, r[mhi])

    C_lo = np.zeros(NB, np.int64)
    C_hi = np.zeros(NB, np.int64)
    for b in range(NB):
        for c in range(N_CORES):
            C_lo[b] = max(C_lo[b], (len(lo_lists[c][b][0]) + P - 1) // P)
            C_hi[b] = max(C_hi[b], (len(hi_lists[c][b][0]) + P - 1) // P)
    C_blk = C_lo + C_hi
    NC = int(C_blk.sum())

    idx_mats, d_mats = [], []
    for c in range(N_CORES):
        idx_flat = np.zeros(NC * P, np.int64)
        dloc = np.full((P, NC), -1.0, np.float32)
        chunk0 = 0
        for b in range(NB):
            for lists, C in ((lo_lists, C_lo[b]), (hi_lists, C_hi[b])):
                s, r = lists[c][b]
                n = len(s)
                base = chunk0 * P
                idx_flat[base : base + n] = s
                j = np.arange(n)
                dloc[j % P, chunk0 + j // P] = r
                chunk0 += C
        idx_mats.append(_wrap16(idx_flat, NC * 8))
        d_mats.append(dloc)

    calls = []
    chunk0 = 0
    for b in range(NB):
        for C, is_hi in ((C_lo[b], False), (C_hi[b], True)):
            left = int(C)
            at = chunk0
            while left > 0:
                k = min(left, MAX_CALL // P)
                calls.append((at, k, is_hi))
                at += k
                left -= k
            chunk0 += int(C)

    xs, dinvs = [], []
    for c in range(N_CORES):
        xc = np.zeros((SHARD, IN_DIM), np.float32)
        dc = np.ones((SHARD,), np.float32)
        sel = old_of_new[c * SHARD : (c + 1) * SHARD]
        real = sel >= 0
        xc[real] = x[sel[real]]
        dc[real] = dinv[sel[real]]
        dw = dc.reshape(NB, P).T.copy()
        xT = np.ascontiguousarray(xc.T.reshape(2, P, SHARD).transpose(1, 0, 2))
        xs.append(xT.reshape(P, 2 * SHARD))
        dinvs.append(dw)

    iota = np.tile(np.arange(P, dtype=np.float32)[None, :], (P, 1))
    ident = np.eye(P, dtype=np.float32)
    b1r = np.tile(np.asarray(b1, np.float32)[None, :], (P, 1))
    b2r = np.tile(np.asarray(b2, np.float32)[None, :], (P, 1))

    in_maps = []
    for c in range(N_CORES):
        in_maps.append(
            {
                "x": xs[c],
                "gidx": idx_mats[c],
                "dmat": d_mats[c],
                "dinv": dinvs[c],
                "w1": np.asarray(W1, np.float32),
                "w2": np.asarray(W2, np.float32),
                "b1r": b1r,
                "b2r": b2r,
                "iota": iota,
                "ident": ident,
            }
        )

    meta = dict(C_blk=C_blk, NC=NC, calls=calls, old_of_new=old_of_new)
    return in_maps, meta


NQ = 4  # SWDGE queues used round-robin for gather calls


def _build_program(meta, table_bf16=False):
    import concourse.mybir as mybir
    import concourse.tile as tile
    from concourse import bacc
    from concourse._compat import get_trn_type

    C_blk = meta["C_blk"]
    NC = meta["NC"]
    calls = meta["calls"]
    C_MAX = int(C_blk.max())
    f32 = mybir.dt.float32
    tdt = mybir.dt.bfloat16 if table_bf16 else f32

    nc = bacc.Bacc(get_trn_type() or "TRN2", num_swdge_queues=NQ)
    x_in = nc.dram_tensor("x", [P, 2 * SHARD], f32, kind="ExternalInput")
    gidx = nc.dram_tensor("gidx", [P, NC * 8], mybir.dt.int16, kind="ExternalInput")
    dmat = nc.dram_tensor("dmat", [P, NC], f32, kind="ExternalInput")
    dinv_in = nc.dram_tensor("dinv", [P, NB], f32, kind="ExternalInput")
    w1_in = nc.dram_tensor("w1", [IN_DIM, HID], f32, kind="ExternalInput")
    w2_in = nc.dram_tensor("w2", [HID, HID], f32, kind="ExternalInput")
    b1_in = nc.dram_tensor("b1r", [P, HID], f32, kind="ExternalInput")
    b2_in = nc.dram_tensor("b2r", [P, HID], f32, kind="ExternalInput")
    iota_in = nc.dram_tensor("iota", [P, P], f32, kind="ExternalInput")
    ident_in = nc.dram_tensor("ident", [P, P], f32, kind="ExternalInput")
    z_out = nc.dram_tensor("z", [SHARD, HID], f32, kind="ExternalOutput")

    cc1_in = nc.dram_tensor("cc1_in", [SHARD, HID], tdt)
    table1 = nc.dram_tensor("table1", [NPAD, HID], tdt, addr_space="Shared")
    cc2_in = nc.dram_tensor("cc2_in", [SHARD, HID], tdt)
    table2 = nc.dram_tensor("table2", [NPAD, HID], tdt, addr_space="Shared")

    rg = [list(range(N_CORES))]

    with tile.TileContext(nc) as tc:
        with (
            tc.tile_pool(name="persist", bufs=1) as pp,
            tc.tile_pool(name="xt", bufs=4) as xtp,
            tc.tile_pool(name="g", bufs=4) as gp,
            tc.tile_pool(name="s", bufs=8) as sp,
            tc.tile_pool(name="ep", bufs=4) as ep,
            tc.tile_pool(name="psum", bufs=2, space="PSUM") as psp,
        ):
            idx_t = pp.tile([P, NC * 8], mybir.dt.int16)
            nc.sync.dma_start(out=idx_t[:], in_=gidx[:])
            dm_t = pp.tile([P, NC], f32)
            nc.sync.dma_start(out=dm_t[:], in_=dmat[:])
            dinv_t = pp.tile([P, NB], f32)
            nc.sync.dma_start(out=dinv_t[:], in_=dinv_in[:])
            iota_t = pp.tile([P, P], f32)
            nc.sync.dma_start(out=iota_t[:], in_=iota_in[:])
            ident_t = pp.tile([P, P], f32)
            nc.sync.dma_start(out=ident_t[:], in_=ident_in[:])
            b1_t = pp.tile([P, HID], f32)
            nc.sync.dma_start(out=b1_t[:], in_=b1_in[:])
            b2_t = pp.tile([P, HID], f32)
            nc.sync.dma_start(out=b2_t[:], in_=b2_in[:])
            w1_t = pp.tile([P, 2 * HID], f32)
            nc.sync.dma_start(
                out=w1_t[:].rearrange("p (k h) -> p k h", k=2),
                in_=w1_in[:].rearrange("(k p) h -> p k h", p=P),
            )
            w2_t = pp.tile([P, HID], f32)
            nc.sync.dma_start(out=w2_t[:], in_=w2_in[:])

            hbuf = pp.tile([P, SHARD], f32)
            x2buf = pp.tile([P, SHARD], f32)
            h2buf = pp.tile([P, SHARD], f32)

            def transform(get_lhsT, w_tiles, out_sbuf, cc_dram):
                nkt = len(w_tiles)
                for t in range(NB):
                    hp = psp.tile([P, HID], f32, tag="hp")
                    for k in range(nkt):
                        nc.tensor.matmul(
                            out=hp[:], lhsT=get_lhsT(t, k), rhs=w_tiles[k],
                            start=(k == 0), stop=(k == nkt - 1),
                        )
                    sl = out_sbuf[:, t * P : (t + 1) * P]
                    nc.vector.tensor_scalar(
                        out=sl, in0=hp[:], scalar1=dinv_t[:, t : t + 1],
                        scalar2=None, op0=mybir.AluOpType.mult,
                    )
                    if table_bf16:
                        hc = xtp.tile([P, HID], tdt, tag="hcast")
                        nc.scalar.copy(out=hc[:], in_=sl)
                        nc.sync.dma_start(
                            out=cc_dram[t * P : (t + 1) * P, :], in_=hc[:]
                        )
                    else:
                        nc.sync.dma_start(
                            out=cc_dram[t * P : (t + 1) * P, :], in_=sl
                        )

            def aggregate(table, hsrc, bias_t, relu, z_dram):
                chunk0 = 0
                ci = 0
                for b in range(NB):
                    Cb = int(C_blk[b])
                    G = gp.tile([P, C_MAX * P], tdt, tag="g")
                    G3 = G[:].rearrange("p (c d) -> p c d", d=P)
                    while ci < len(calls) and calls[ci][0] < chunk0 + Cb:
                        at, k, is_hi = calls[ci]
                        n = k * P
                        src = table[HI_BASE:, :] if is_hi else table[:, :]
                        nc.gpsimd.dma_gather(
                            G3[:, at - chunk0 : at - chunk0 + k, :],
                            src,
                            idx_t[:, at * 8 : at * 8 + n // 16],
                            n, n, HID,
                            queue_num=ci % NQ,
                        )
                        ci += 1
                    acc = psp.tile([P, HID], f32, tag="acc")
                    for i in range(Cb):
                        S = sp.tile([P, P], tdt, tag="S")
                        nc.vector.tensor_tensor(
                            out=S[:], in0=iota_t[:],
                            in1=dm_t[
                                :, chunk0 + i : chunk0 + i + 1
                            ].to_broadcast([P, P]),
                            op=mybir.AluOpType.is_equal,
                        )
                        nc.tensor.matmul(
                            out=acc[:], lhsT=S[:], rhs=G3[:, i, :],
                            start=(i == 0), stop=(i == Cb - 1),
                        )
                    t1 = ep.tile([P, HID], f32, tag="t1")
                    nc.vector.tensor_tensor(
                        out=t1[:], in0=acc[:],
                        in1=hsrc[:, b * P : (b + 1) * P],
                        op=mybir.AluOpType.add,
                    )
                    t2 = ep.tile([P, HID], f32, tag="t2")
                    nc.vector.scalar_tensor_tensor(
                        out=t2[:], in0=t1[:],
                        scalar=dinv_t[:, b : b + 1], in1=bias_t[:],
                        op0=mybir.AluOpType.mult, op1=mybir.AluOpType.add,
                    )
                    if relu:
                        nc.scalar.activation(
                            out=x2buf[:, b * P : (b + 1) * P], in_=t2[:],
                            func=mybir.ActivationFunctionType.Relu,
                        )
                    else:
                        nc.sync.dma_start(
                            out=z_dram[b * P : (b + 1) * P, :], in_=t2[:]
                        )
                    chunk0 += Cb

            # layer 1
            xT_t = pp.tile([P, 2 * SHARD], f32)
            nc.sync.dma_start(out=xT_t[:], in_=x_in[:])
            xT3 = xT_t[:].rearrange("p (k n) -> p k n", k=2)
            transform(
                lambda t, k: xT3[:, k, t * P : (t + 1) * P],
                [w1_t[:, 0:HID], w1_t[:, HID : 2 * HID]], hbuf, cc1_in,
            )
            nc.gpsimd.collective_compute(
                "AllGather", mybir.AluOpType.bypass, replica_groups=rg,
                ins=[cc1_in[:]], outs=[table1[:]],
            )
            aggregate(table1, hbuf, b1_t, True, None)

            # layer 2
            def l2_lhsT(t, k):
                tp = psp.tile([P, P], f32, tag="tp")
                nc.tensor.transpose(
                    out=tp[:], in_=x2buf[:, t * P : (t + 1) * P],
                    identity=ident_t[:],
                )
                xT = xtp.tile([P, P], f32, tag="xT")
                nc.scalar.copy(out=xT[:], in_=tp[:])
                return xT[:]

            transform(l2_lhsT, [w2_t[:]], h2buf, cc2_in)
            nc.gpsimd.collective_compute(
                "AllGather", mybir.AluOpType.bypass, replica_groups=rg,
                ins=[cc2_in[:]], outs=[table2[:]],
            )
            aggregate(table2, h2buf, b2_t, False, z_out)

    nc.compile()
    return nc


def kernel(x, edge_index, W1, b1, W2, b2):
    global LAST_EXEC_NS, LAST_RESULT
    from concourse.bass_utils import run_bass_kernel_spmd

    trace = bool(int(os.environ.get("GCN_TRACE", "0")))
    if trace:
        try:  # NTFF profiling shim (axon images lack antenv.axon_hooks)
            _install_ntff_shim()
        except Exception:
            trace = False

    table_bf16 = bool(int(os.environ.get("GCN_BF16", "1")))
    in_maps, meta = _host_prep(x, edge_index, W1, b1, W2, b2)
    nc = _build_program(meta, table_bf16=table_bf16)
    res = run_bass_kernel_spmd(
        nc, in_maps, core_ids=list(range(N_CORES)), trace=trace
    )
    LAST_EXEC_NS = res.exec_time_ns
    LAST_RESULT = res

    old_of_new = meta["old_of_new"]
    z = np.zeros((N_NODES, HID), np.float32)
    for c in range(N_CORES):
        zc = np.asarray(res.results[c]["z"])
        sel = old_of_new[c * SHARD : (c + 1) * SHARD]
        real = sel >= 0
        z[sel[real]] = zc[real]
    return z


def _install_ntff_shim():
    import contextlib
    import ctypes
    import sys
    import types

    if "antenv.axon_hooks" in sys.modules:
        return
    lib = ctypes.CDLL("/opt/axon/libaxon_pjrt.so")
    if not hasattr(lib, "axon_start_nrt_profile"):
        raise RuntimeError("no profile symbols")
    lib.axon_start_nrt_profile.argtypes = [
        ctypes.POINTER(ctypes.c_int64),
        ctypes.c_size_t,
    ]
    lib.axon_start_nrt_profile.restype = ctypes.c_int64
    lib.axon_stop_nrt_profile.argtypes = [ctypes.c_char_p]
    lib.axon_stop_nrt_profile.restype = ctypes.c_int64

    @contextlib.contextmanager
    def _hook(output_dir, device_ids):
        import jax

        jax.devices()
        if device_ids:
            ids = (ctypes.c_int64 * len(device_ids))(*device_ids)
            rc = lib.axon_start_nrt_profile(ids, len(device_ids))
        else:
            rc = lib.axon_start_nrt_profile(None, 0)
        if rc != 0:
            raise RuntimeError(f"axon_start_nrt_profile rc={rc}")
        try:
            yield
        finally:
            lib.axon_stop_nrt_profile(str(output_dir).encode())

    mod = types.ModuleType("antenv.axon_hooks")
    mod.get_axon_ntff_profile_hook = lambda: _hook
    mod.set_axon_ntff_profile_hook = lambda h: None
    sys.modules["antenv.axon_hooks"] = mod
    import antenv

    antenv.axon_hooks = mod

